# revision 19
# baseline (speedup 1.0000x reference)
"""Trainium2 Bass kernel for nn_AttentionLayer_57930518888709.

reference:
    h = relu(x @ W1 + b1); h = relu(h @ W2 + b2); logits = h @ W3 + b3
    tns = logits*m - 999*(1-m); out = softmax(tns, axis=1)       # [B, N, 1]

Shapes: x [64, 4096, 64] f32, mask [64, 4096] i32, W1 [64,128], W2 [128,128],
W3 [128,1].  Pure data parallel over batch: 8 batches per core on 8 cores.

Per-core layout strategy:
  - x is host-transposed into 4 "pair" tiles [128, 4096]: rows 0-63 are the
    even batch's 64 features, rows 64-127 the odd batch's.  L1 (K=64) then
    runs as row-tiled matmul pairs that use the full 128x128 PE array.
  - L3 (H2 -> 1) is computed as 32 accumulating matmuls per 32-row column
    group, with host-built lhsT blocks that place w3 in column m of a
    [128, 32] zero matrix.  Row 32j + m of the shared PSUM logits tile
    receives w3 . h2_chunk; all other rows accumulate += 0.  This lands the
    64*4096 logits directly in a [128 partitions, 256] PSUM tile whose
    partition p = 16*batch + chunk(256 tokens) -- the exact layout the
    masked softmax wants, with no transposes anywhere.  The four column
    groups are issued adjacently so their streams overlap in the array.
  - matmul inputs are bf16 (x/W rounded on host, h1/h2 rounded by the relu
    drains); PSUM accumulation stays fp32.  Measured end-to-end error vs the
    fp32 reference is ~1e-3 relative, dominated by the bf16 rounding.
  - softmax needs no max-subtraction: logits are O(1) (tiny MLP outputs), and
    masked lanes compute exp(0 - 999) which underflows to exactly 0.0, the
    same value the reference's exp(-999 - max) produces.

Performance notes (measured via neuron-profile NTFF on axon trn2):
  - ~67.6 us HW exec (vs ~111 us for the first correct fp32r version).
  - Per-core work: 4.2 MB x DMA (bf16), 128 L1/L2 matmuls (N=512, ~215 ns
    warm) + 128 col-group-packed L3 matmuls (4 concurrent, ~110 ns/wave),
    and 64 PSUM->SBUF relu drains ([128,1024], ~1.05-1.2 us) split between
    the Scalar and Vector engines by a greedy cost balance.  The drains are
    the binding resource (~37 us/engine); fp32 PSUM reads run at 1
    elem/lane/cycle on both engines and DMA cannot touch PSUM, so this is
    structural for a dense kernel.
  - fp32 matmul is 4 cycles/column on TRN2 and fp32r requires dst partition
    0 (no column tiling), which is why the matmul path is bf16.
"""

import os
import sys

for _p in ("/opt/trn_rl_repo", "/root/.axon_site/_ro/trn_rl_repo"):
    if os.path.isdir(_p) and _p not in sys.path:
        sys.path.insert(0, _p)

import ml_dtypes
import numpy as np

import concourse.mybir as mybir  # noqa: E402
import concourse.tile as tile  # noqa: E402
from concourse import bacc  # noqa: E402
from concourse.bass_utils import run_bass_kernel_spmd  # noqa: E402

F32 = mybir.dt.float32
BF16 = mybir.dt.bfloat16
AF = mybir.ActivationFunctionType
ALU = mybir.AluOpType

B, N, F, H1, H2 = 64, 4096, 64, 128, 128
NCORES = 8
BPC = B // NCORES          # 8 batches per core
NPAIR = BPC // 2           # 4 batch-pairs per core
NTT = N // 512             # 8 token-tiles of 512 per batch
NEG = 999.0

# filled by kernel(); test.py reads exec_time_ns / trace path from here
last_results = None


def _build_program(b3val: float, has_b1: bool, has_b2: bool):
    nc = bacc.Bacc(
        "TRN2", target_bir_lowering=False, debug=False, num_devices=NCORES
    )

    xp_d = nc.dram_tensor("xp", [NPAIR, 128, N], BF16, kind="ExternalInput")
    # wpack: [w1s | w2 | w3s] along free dim, bf16
    wp_d = nc.dram_tensor("wpack", [128, 1280], BF16, kind="ExternalInput")
    # cpack: [b1c | b2c | negc | sel(8) | sel2(rows 0-7, 128) | maskf(256)]
    cp_d = nc.dram_tensor("cpack", [128, 395], F32, kind="ExternalInput")
    out_d = nc.dram_tensor("out", [128, 256], F32, kind="ExternalOutput")

    with tile.TileContext(nc) as tc:
        with (
            tc.tile_pool(name="consts", bufs=1) as cpool,
            tc.tile_pool(name="xpool", bufs=1) as xpool,
            tc.tile_pool(name="hpool", bufs=1) as hpool,
            tc.tile_pool(name="spool", bufs=1) as spool,
            tc.tile_pool(name="mmps", bufs=3, space="PSUM") as mmps,
            tc.tile_pool(name="lgps", bufs=1, space="PSUM") as lgps,
        ):
            # --- packed constants on the ACT HWDGE ring (parallel w/ x) ----
            wp = cpool.tile([128, 1280], BF16, name="wp_sb")
            nc.scalar.dma_start(wp[:], wp_d[:])
            cp = cpool.tile([128, 395], F32, name="cp_sb")
            nc.scalar.dma_start(cp[:], cp_d[:])
            w1s = wp[:, 0:128]
            w2 = wp[:, 128:256]
            w3s = wp[:, 256:1280]
            b1c = cp[:, 0:1]
            b2c = cp[:, 1:2]
            negc = cp[:, 2:3]
            sel = cp[:, 3:11]
            sel2 = cp[0:8, 11:139]
            mf = cp[:, 139:395]

            # x: 4 pair tiles [128, 4096] bf16, DMAed as half-chunks,
            # j-major so every pair's early tokens land first.
            xts = []
            for j in range(NPAIR):
                xt = xpool.tile([128, N], BF16, name=f"x_{j}", tag=f"x{j}")
                xts.append(xt)
            xoff = 0
            for h, chw in enumerate((1024, 1024, 2048)):
                for j in range(NPAIR):
                    eng = nc.sync if j % 2 == 0 else nc.gpsimd
                    eng.dma_start(
                        xts[j][:, xoff : xoff + chw],
                        xp_d[j, :, xoff : xoff + chw],
                    )
                xoff += chw

            # logits accumulator: partition p = 16*batch + chunk, free = 256
            lg = lgps.tile([128, 256], F32, name="lg_ps", tag="lg")

            # greedy ACT/DVE balance using measured per-op costs
            eng_load = {"act": 0.0, "dve": 0.0}
            ENG_COST = {"act": 1258.0, "dve": 1453.0}

            def drain(dst, src, bias, has_bias):
                """relu(src + bias) -> dst, PSUM -> SBUF (bf16 out)."""
                eng = min(eng_load, key=lambda e: eng_load[e] + ENG_COST[e])
                eng_load[eng] += ENG_COST[eng]
                if eng == "act":
                    if has_bias:
                        nc.scalar.activation(dst, src, AF.Relu, bias=bias)
                    else:
                        nc.scalar.activation(dst, src, AF.Relu)
                else:
                    if has_bias:
                        nc.vector.tensor_scalar(
                            dst, src, bias, 0.0, op0=ALU.add, op1=ALU.max
                        )
                    else:
                        nc.vector.tensor_scalar_max(dst, src, 0.0)

            # --- main loop (L3 software-pipelined one iteration back) ------
            prev_h2 = None

            def l3_waves(tt, h2s):
                for w in range(4):
                    bp, cp = w // 2, w % 2
                    m = bp * 16 + 2 * tt + cp
                    for j in range(NPAIR):
                        off = bp * 512 + cp * 256
                        nc.tensor.matmul(
                            lg[32 * j : 32 * j + 32, :],
                            w3s[:, 32 * m : 32 * m + 32],
                            h2s[j][:, off : off + 256],
                            start=(tt == 0 and w == 0),
                            stop=(tt == NTT - 1 and w == 3),
                            tile_position=(0, 32 * j),
                            skip_group_check=True,
                        )

            for tt in range(NTT):
                ts = tt * 512
                h1ts = [None] * NPAIR
                h2ts = [None] * NPAIR

                def mm_l1(j):
                    ha = mmps.tile([128, 1024], F32, name="ha", tag="ps")
                    nc.tensor.matmul(
                        ha[:, 0:512], w1s[0:64, :], xts[j][0:64, ts : ts + 512]
                    )
                    nc.tensor.matmul(
                        ha[:, 512:1024],
                        w1s[64:128, :],
                        xts[j][64:128, ts : ts + 512],
                    )
                    return ha

                def mm_l2(j):
                    hb = mmps.tile([128, 1024], F32, name="hb", tag="ps")
                    nc.tensor.matmul(hb[:, 0:512], w2[:], h1ts[j][:, 0:512])
                    nc.tensor.matmul(
                        hb[:, 512:1024], w2[:], h1ts[j][:, 512:1024]
                    )
                    return hb

                def d1(j, ha):
                    h1t = hpool.tile(
                        [128, 1024], BF16, name=f"h1_{j}", tag="h1", bufs=6
                    )
                    drain(h1t[:], ha[:], b1c[:], has_b1)
                    h1ts[j] = h1t

                def d2(j, hb):
                    h2t = hpool.tile(
                        [128, 1024], BF16, name=f"h2_{j}", tag="h2", bufs=10
                    )
                    drain(h2t[:], hb[:], b2c[:], has_b2)
                    h2ts[j] = h2t

                # round-robin so PE / DVE / ACT stay concurrently fed;
                # the previous iteration's L3 waves slot into the drain phase
                ha0 = mm_l1(0)
                ha1 = mm_l1(1)
                d1(0, ha0)
                ha2 = mm_l1(2)
                d1(1, ha1)
                if prev_h2 is not None:
                    l3_waves(tt - 1, prev_h2)
                hb0 = mm_l2(0)
                ha3 = mm_l1(3)
                d1(2, ha2)
                hb1 = mm_l2(1)
                d2(0, hb0)
                d1(3, ha3)
                hb2 = mm_l2(2)
                d2(1, hb1)
                hb3 = mm_l2(3)
                d2(2, hb2)
                d2(3, hb3)
                prev_h2 = h2ts

            l3_waves(NTT - 1, prev_h2)

            # --- masked softmax over each batch's 4096 logits --------------
            # u = (logits + (999 + b3)) * m   -> masked lanes are exactly 0
            u = spool.tile([128, 256], F32, name="u_sb")
            nc.vector.scalar_tensor_tensor(
                u[:], lg[:], float(NEG + b3val), mf[:], op0=ALU.add, op1=ALU.mult
            )
            # e = exp(u - 999): unmasked = exp(logits + b3), masked = 0.0
            # accum_out gives the per-partition sum of e for free
            e = spool.tile([128, 256], F32, name="e_sb")
            s1t = spool.tile([128, 1], F32, name="s1_sb")
            nc.scalar.activation(
                e[:], u[:], AF.Exp, bias=negc[:], scale=1.0, accum_out=s1t[:]
            )

            s8 = lgps.tile([BPC, 1], F32, name="s8_ps", tag="lg")
            nc.tensor.matmul(s8[:], sel[:], s1t[:])
            r8 = spool.tile([BPC, 1], F32, name="r8_sb")
            nc.vector.reciprocal(r8[:], s8[:])
            rb = lgps.tile([128, 1], F32, name="rb_ps", tag="lg")
            nc.tensor.matmul(rb[:], sel2[:], r8[:])

            outt = spool.tile([128, 256], F32, name="out_sb")
            nc.vector.tensor_scalar_mul(outt[:], e[:], rb[:])
            nc.sync.dma_start(out_d[:], outt[:])

    nc.compile()
    return nc


_program_cache = {}


def _get_program(b3val: float, has_b1: bool, has_b2: bool):
    key = (round(float(b3val), 12), has_b1, has_b2)
    if key not in _program_cache:
        _program_cache[key] = _build_program(b3val, has_b1, has_b2)
    return _program_cache[key]


def _host_inputs(x, mask, W1, b1, W2, b2, W3, b3):
    """Build the per-core in_maps."""
    x = np.asarray(x, dtype=np.float32)
    mask = np.asarray(mask)
    W1 = np.asarray(W1, dtype=np.float32)
    W2 = np.asarray(W2, dtype=np.float32)
    W3 = np.asarray(W3, dtype=np.float32)
    b1 = np.asarray(b1, dtype=np.float32)
    b2 = np.asarray(b2, dtype=np.float32)

    bf = ml_dtypes.bfloat16
    w1s = np.concatenate([W1, W1], axis=0)                       # [128, 128]
    w3s = np.zeros((H2, 32 * 32), dtype=np.float32)
    for m in range(32):
        w3s[:, 32 * m + m] = W3[:, 0]
    wpack = np.concatenate([w1s, W2, w3s], axis=1).astype(bf)    # [128, 1280]
    sel = np.zeros((128, BPC), dtype=np.float32)
    sel2r = np.zeros((128, 128), dtype=np.float32)
    for p in range(128):
        sel[p, p // 16] = 1.0
        sel2r[p // 16, p] = 1.0                                  # rows 0-7 used

    # [core, pair, b'(2), N, F] -> [core, pair, b'*F(128 rows), N]
    xps = np.ascontiguousarray(
        x.reshape(NCORES, NPAIR, 2, N, F).transpose(0, 1, 2, 4, 3)
    ).reshape(NCORES, NPAIR, 128, N).astype(bf)
    maskf = (
        mask.astype(np.float32).reshape(NCORES, BPC, 16, 256).reshape(NCORES, 128, 256)
    )

    base = np.zeros((128, 139), dtype=np.float32)
    base[:, 0:1] = b1.reshape(H1, 1)
    base[:, 1:2] = b2.reshape(H2, 1)
    base[:, 2:3] = -NEG
    base[:, 3:11] = sel
    base[:, 11:139] = sel2r

    in_maps = []
    for c in range(NCORES):
        cpack = np.concatenate([base, maskf[c]], axis=1)         # [128, 395]
        in_maps.append({"wpack": wpack, "cpack": cpack, "xp": xps[c]})
    return in_maps


def kernel(x, mask, W1, b1, W2, b2, W3, b3):
    global last_results
    b3val = float(np.asarray(b3).reshape(-1)[0])
    b1a = np.asarray(b1, dtype=np.float32)
    b2a = np.asarray(b2, dtype=np.float32)
    nc = _get_program(b3val, bool(np.any(b1a)), bool(np.any(b2a)))
    in_maps = _host_inputs(x, mask, W1, b1, W2, b2, W3, b3)
    res = run_bass_kernel_spmd(nc, in_maps, core_ids=list(range(NCORES)))
    last_results = res
    outs = [res.results[c]["out"].reshape(BPC, N) for c in range(NCORES)]
    full = np.concatenate(outs, axis=0)                          # [64, 4096]
    return full[..., None].astype(np.float32)


# revision 21
# speedup vs baseline: 1.0062x; 1.0062x over previous
"""Trainium2 Bass kernel for nn_AttentionLayer_57930518888709.

reference:
    h = relu(x @ W1 + b1); h = relu(h @ W2 + b2); logits = h @ W3 + b3
    tns = logits*m - 999*(1-m); out = softmax(tns, axis=1)       # [B, N, 1]

Shapes: x [64, 4096, 64] f32, mask [64, 4096] i32, W1 [64,128], W2 [128,128],
W3 [128,1].  Pure data parallel over batch: 8 batches per core on 8 cores.

Per-core layout strategy:
  - x is host-transposed into 4 "pair" tiles [128, 4096]: rows 0-63 are the
    even batch's 64 features, rows 64-127 the odd batch's.  L1 (K=64) then
    runs as row-tiled matmul pairs that use the full 128x128 PE array.
  - L3 (H2 -> 1) is computed as 32 accumulating matmuls per 32-row column
    group, with host-built lhsT blocks that place w3 in column m of a
    [128, 32] zero matrix.  Row 32j + m of the shared PSUM logits tile
    receives w3 . h2_chunk; all other rows accumulate += 0.  This lands the
    64*4096 logits directly in a [128 partitions, 256] PSUM tile whose
    partition p = 16*batch + chunk(256 tokens) -- the exact layout the
    masked softmax wants, with no transposes anywhere.  The four column
    groups are issued adjacently so their streams overlap in the array.
  - matmul inputs are bf16 (x/W rounded on host, h1/h2 rounded by the relu
    drains); PSUM accumulation stays fp32.  Measured end-to-end error vs the
    fp32 reference is ~1e-3 relative, dominated by the bf16 rounding.
  - softmax needs no max-subtraction: logits are O(1) (tiny MLP outputs), and
    masked lanes compute exp(0 - 999) which underflows to exactly 0.0, the
    same value the reference's exp(-999 - max) produces.

Performance notes (measured via neuron-profile NTFF on axon trn2):
  - ~67.6 us HW exec (vs ~111 us for the first correct fp32r version).
  - Per-core work: 4.2 MB x DMA (bf16), 128 L1/L2 matmuls (N=512, ~215 ns
    warm) + 128 col-group-packed L3 matmuls (4 concurrent, ~110 ns/wave),
    and 64 PSUM->SBUF relu drains ([128,1024], ~1.05-1.2 us) split between
    the Scalar and Vector engines by a greedy cost balance.  The drains are
    the binding resource (~37 us/engine); fp32 PSUM reads run at 1
    elem/lane/cycle on both engines and DMA cannot touch PSUM, so this is
    structural for a dense kernel.
  - fp32 matmul is 4 cycles/column on TRN2 and fp32r requires dst partition
    0 (no column tiling), which is why the matmul path is bf16.
"""

import os
import sys

for _p in ("/opt/trn_rl_repo", "/root/.axon_site/_ro/trn_rl_repo"):
    if os.path.isdir(_p) and _p not in sys.path:
        sys.path.insert(0, _p)

import ml_dtypes
import numpy as np

import concourse.mybir as mybir  # noqa: E402
import concourse.tile as tile  # noqa: E402
from concourse import bacc  # noqa: E402
from concourse.bass_utils import run_bass_kernel_spmd  # noqa: E402

F32 = mybir.dt.float32
BF16 = mybir.dt.bfloat16
AF = mybir.ActivationFunctionType
ALU = mybir.AluOpType

B, N, F, H1, H2 = 64, 4096, 64, 128, 128
NCORES = 8
BPC = B // NCORES          # 8 batches per core
NPAIR = BPC // 2           # 4 batch-pairs per core
NTT = N // 512             # 8 token-tiles of 512 per batch
NEG = 999.0

# filled by kernel(); test.py reads exec_time_ns / trace path from here
last_results = None


def _build_program(b3val: float, has_b1: bool, has_b2: bool):
    nc = bacc.Bacc(
        "TRN2", target_bir_lowering=False, debug=False, num_devices=NCORES
    )

    xp_d = nc.dram_tensor("xp", [NPAIR, 128, N], BF16, kind="ExternalInput")
    # wpack: [w1s | w2 | w3s] along free dim, bf16
    wp_d = nc.dram_tensor("wpack", [128, 1280], BF16, kind="ExternalInput")
    # cpack: [b1c | b2c | negc | sel(8) | sel2(rows 0-7, 128) | maskf(256)]
    cp_d = nc.dram_tensor("cpack", [128, 395], F32, kind="ExternalInput")
    out_d = nc.dram_tensor("out", [128, 256], F32, kind="ExternalOutput")

    with tile.TileContext(nc) as tc:
        with (
            tc.tile_pool(name="consts", bufs=1) as cpool,
            tc.tile_pool(name="xpool", bufs=1) as xpool,
            tc.tile_pool(name="hpool", bufs=1) as hpool,
            tc.tile_pool(name="spool", bufs=1) as spool,
            tc.tile_pool(name="mmps", bufs=3, space="PSUM") as mmps,
            tc.tile_pool(name="lgps", bufs=1, space="PSUM") as lgps,
        ):
            # --- packed constants on the ACT HWDGE ring (parallel w/ x) ----
            wp = cpool.tile([128, 1280], BF16, name="wp_sb")
            nc.scalar.dma_start(wp[:], wp_d[:])
            cp = cpool.tile([128, 395], F32, name="cp_sb")
            nc.scalar.dma_start(cp[:], cp_d[:])
            w1s = wp[:, 0:128]
            w2 = wp[:, 128:256]
            w3s = wp[:, 256:1280]
            b1c = cp[:, 0:1]
            b2c = cp[:, 1:2]
            negc = cp[:, 2:3]
            sel = cp[:, 3:11]
            sel2 = cp[0:8, 11:139]
            mf = cp[:, 139:395]

            # x: 4 pair tiles [128, 4096] bf16, DMAed as half-chunks,
            # j-major so every pair's early tokens land first.
            xts = []
            for j in range(NPAIR):
                xt = xpool.tile([128, N], BF16, name=f"x_{j}", tag=f"x{j}")
                xts.append(xt)
            xoff = 0
            for h, chw in enumerate((1024, 1024, 2048)):
                for j in range(NPAIR):
                    eng = nc.sync if j % 2 == 0 else nc.gpsimd
                    eng.dma_start(
                        xts[j][:, xoff : xoff + chw],
                        xp_d[j, :, xoff : xoff + chw],
                    )
                xoff += chw

            # logits accumulator: partition p = 16*batch + chunk, free = 256
            lg = lgps.tile([128, 256], F32, name="lg_ps", tag="lg")

            # greedy ACT/DVE balance using measured per-op costs
            eng_load = {"act": 0.0, "dve": 0.0}
            ENG_COST = {"act": 1258.0, "dve": 1453.0}

            def drain(dst, src, bias, has_bias):
                """relu(src + bias) -> dst, PSUM -> SBUF (bf16 out)."""
                eng = min(eng_load, key=lambda e: eng_load[e] + ENG_COST[e])
                eng_load[eng] += ENG_COST[eng]
                if eng == "act":
                    if has_bias:
                        nc.scalar.activation(dst, src, AF.Relu, bias=bias)
                    else:
                        nc.scalar.activation(dst, src, AF.Relu)
                else:
                    if has_bias:
                        nc.vector.tensor_scalar(
                            dst, src, bias, 0.0, op0=ALU.add, op1=ALU.max
                        )
                    else:
                        nc.vector.tensor_scalar_max(dst, src, 0.0)

            # --- main loop (L3 software-pipelined one iteration back) ------
            prev_h2 = None

            def l3_waves(tt, h2s):
                for w in range(4):
                    bp, cp = w // 2, w % 2
                    m = bp * 16 + 2 * tt + cp
                    for j in range(NPAIR):
                        off = bp * 512 + cp * 256
                        nc.tensor.matmul(
                            lg[32 * j : 32 * j + 32, :],
                            w3s[:, 32 * m : 32 * m + 32],
                            h2s[j][:, off : off + 256],
                            start=(tt == 0 and w == 0),
                            stop=(tt == NTT - 1 and w == 3),
                            tile_position=(0, 32 * j),
                            skip_group_check=True,
                        )

            for tt in range(NTT):
                ts = tt * 512
                h1ts = [None] * NPAIR
                h2ts = [None] * NPAIR

                def mm_l1(j):
                    ha = mmps.tile([128, 1024], F32, name="ha", tag="ps")
                    nc.tensor.matmul(
                        ha[:, 0:512], w1s[0:64, :], xts[j][0:64, ts : ts + 512]
                    )
                    nc.tensor.matmul(
                        ha[:, 512:1024],
                        w1s[64:128, :],
                        xts[j][64:128, ts : ts + 512],
                    )
                    return ha

                def mm_l2(j):
                    hb = mmps.tile([128, 1024], F32, name="hb", tag="ps")
                    nc.tensor.matmul(hb[:, 0:512], w2[:], h1ts[j][:, 0:512])
                    nc.tensor.matmul(
                        hb[:, 512:1024], w2[:], h1ts[j][:, 512:1024]
                    )
                    return hb

                def d1(j, ha):
                    h1t = hpool.tile(
                        [128, 1024], BF16, name=f"h1_{j}", tag="h1", bufs=6
                    )
                    drain(h1t[:], ha[:], b1c[:], has_b1)
                    h1ts[j] = h1t

                def d2(j, hb):
                    h2t = hpool.tile(
                        [128, 1024], BF16, name=f"h2_{j}", tag="h2", bufs=10
                    )
                    drain(h2t[:], hb[:], b2c[:], has_b2)
                    h2ts[j] = h2t

                # round-robin so PE / DVE / ACT stay concurrently fed;
                # the previous iteration's L3 waves slot into the drain phase
                ha0 = mm_l1(0)
                ha1 = mm_l1(1)
                d1(0, ha0)
                ha2 = mm_l1(2)
                d1(1, ha1)
                if prev_h2 is not None:
                    l3_waves(tt - 1, prev_h2)
                hb0 = mm_l2(0)
                ha3 = mm_l1(3)
                d1(2, ha2)
                hb1 = mm_l2(1)
                d2(0, hb0)
                d1(3, ha3)
                hb2 = mm_l2(2)
                d2(1, hb1)
                hb3 = mm_l2(3)
                d2(2, hb2)
                d2(3, hb3)
                prev_h2 = h2ts

            l3_waves(NTT - 1, prev_h2)

            # --- masked softmax over each batch's 4096 logits --------------
            # u = (logits + (999 + b3)) * m   -> masked lanes are exactly 0
            u = spool.tile([128, 256], F32, name="u_sb")
            nc.vector.scalar_tensor_tensor(
                u[:], lg[:], float(NEG + b3val), mf[:], op0=ALU.add, op1=ALU.mult
            )
            # e = exp(u - 999): unmasked = exp(logits + b3), masked = 0.0
            # accum_out gives the per-partition sum of e for free
            e = spool.tile([128, 256], F32, name="e_sb")
            s1t = spool.tile([128, 1], F32, name="s1_sb")
            nc.scalar.activation(
                e[:], u[:], AF.Exp, bias=negc[:], scale=1.0, accum_out=s1t[:]
            )

            s8 = lgps.tile([BPC, 1], F32, name="s8_ps", tag="lg")
            nc.tensor.matmul(s8[:], sel[:], s1t[:])
            r8 = spool.tile([BPC, 1], F32, name="r8_sb")
            nc.vector.reciprocal(r8[:], s8[:])
            rb = lgps.tile([128, 1], F32, name="rb_ps", tag="lg")
            nc.tensor.matmul(rb[:], sel2[:], r8[:])

            outt = spool.tile([128, 256], F32, name="out_sb")
            nc.vector.tensor_scalar_mul(outt[:], e[:], rb[:])
            nc.sync.dma_start(out_d[:], outt[:])

    nc.compile()
    return nc


_program_cache = {}


def _get_program(b3val: float, has_b1: bool, has_b2: bool):
    key = (round(float(b3val), 12), has_b1, has_b2)
    if key not in _program_cache:
        _program_cache[key] = _build_program(b3val, has_b1, has_b2)
    return _program_cache[key]


def _host_inputs(x, mask, W1, b1, W2, b2, W3, b3):
    """Build the per-core in_maps."""
    x = np.asarray(x, dtype=np.float32)
    mask = np.asarray(mask)
    W1 = np.asarray(W1, dtype=np.float32)
    W2 = np.asarray(W2, dtype=np.float32)
    W3 = np.asarray(W3, dtype=np.float32)
    b1 = np.asarray(b1, dtype=np.float32)
    b2 = np.asarray(b2, dtype=np.float32)

    bf = ml_dtypes.bfloat16
    w1s = np.concatenate([W1, W1], axis=0)                       # [128, 128]
    w3s = np.zeros((H2, 32 * 32), dtype=np.float32)
    for m in range(32):
        w3s[:, 32 * m + m] = W3[:, 0]
    wpack = np.concatenate([w1s, W2, w3s], axis=1).astype(bf)    # [128, 1280]
    sel = np.zeros((128, BPC), dtype=np.float32)
    sel2r = np.zeros((128, 128), dtype=np.float32)
    for p in range(128):
        sel[p, p // 16] = 1.0
        sel2r[p // 16, p] = 1.0                                  # rows 0-7 used

    # [core, pair, b'(2), N, F] -> [core, pair, b'*F(128 rows), N]
    xps = np.ascontiguousarray(
        x.reshape(NCORES, NPAIR, 2, N, F).transpose(0, 1, 2, 4, 3)
    ).reshape(NCORES, NPAIR, 128, N).astype(bf)
    maskf = (
        mask.astype(np.float32).reshape(NCORES, BPC, 16, 256).reshape(NCORES, 128, 256)
    )

    base = np.zeros((128, 139), dtype=np.float32)
    base[:, 0:1] = b1.reshape(H1, 1)
    base[:, 1:2] = b2.reshape(H2, 1)
    base[:, 2:3] = -NEG
    base[:, 3:11] = sel
    base[:, 11:139] = sel2r

    in_maps = []
    for c in range(NCORES):
        cpack = np.concatenate([base, maskf[c]], axis=1)         # [128, 395]
        in_maps.append({"wpack": wpack, "cpack": cpack, "xp": xps[c]})
    return in_maps


def kernel(x, mask, W1, b1, W2, b2, W3, b3):
    global last_results
    b3val = float(np.asarray(b3).reshape(-1)[0])
    b1a = np.asarray(b1, dtype=np.float32)
    b2a = np.asarray(b2, dtype=np.float32)
    nc = _get_program(b3val, bool(np.any(b1a)), bool(np.any(b2a)))
    in_maps = _host_inputs(x, mask, W1, b1, W2, b2, W3, b3)
    res = run_bass_kernel_spmd(nc, in_maps, core_ids=list(range(NCORES)))
    last_results = res
    outs = [res.results[c]["out"].reshape(BPC, N) for c in range(NCORES)]
    full = np.concatenate(outs, axis=0)                          # [64, 4096]
    return full[..., None].astype(np.float32)
